# revision 10
# baseline (speedup 1.0000x reference)
"""Causal self-attention (dense transformer block) on 8 Trainium2 NeuronCores.

Sharding: 2 batch groups x 4 cores. Within a group each core owns 4 heads
(tensor parallel) for qkv+attention, then an AllGather of y^T inside the
group lets each core compute a disjoint 256-column slice of the output
projection (column-parallel proj => no rank-dependent addressing needed).

x:      [2, 2048, 1024] f32
w_qkv:  [3072, 1024]    f32   (rows: q 0:1024, k 1024:2048, v 2048:3072)
w_proj: [1024, 1024]    f32
out:    [2, 2048, 1024] f32
"""

import sys

if "/opt/trn_rl_repo" not in sys.path:
    sys.path.insert(0, "/opt/trn_rl_repo")

from contextlib import ExitStack

import numpy as np

import concourse.bass as bass
import concourse.mybir as mybir
import concourse.tile as tile
from concourse.bass_utils import run_bass_kernel_spmd
from concourse.vector_clock import ScopedClock

F32 = mybir.dt.float32
F32R = mybir.dt.float32r
EXP = mybir.ActivationFunctionType.Exp

N_EMBD = 1024
SEQ = 2048
BSZ = 2
N_CORES = 8
GROUP = 4                 # cores per batch group
HEADS_PER_CORE = 4
HEAD_DIM = 64
CH = HEADS_PER_CORE * HEAD_DIM   # 256 channels per core
KT = N_EMBD // 128        # 8 contraction tiles over embd
SEQ_T = SEQ // 128        # 16 seq tiles
QCH = 512                 # q chunk (free dim of S^T matmuls)
NEG = -1.0e30
SCALE = 1.0 / 8.0         # 1/sqrt(64)


_ENGINE_OK = {
    mybir.EngineType.PE,
    mybir.EngineType.DVE,
    mybir.EngineType.Activation,
    mybir.EngineType.Pool,
    mybir.EngineType.SP,
}


class SafeTileContext(tile.TileContext):
    """This walrus build accepts only a single sync-wait per TPB engine
    instruction; Tile's add_semaphores attaches every required wait to the
    consuming instruction. Spill excess waits onto same-engine NOPs placed
    immediately before the instruction (engine program order preserves
    semantics). DMACopy is exempt (DGE-ring lowering handles multi-wait)."""

    def _spill_waits(self, inst):
        si = inst.sync_info
        if si is None or len(si.on_wait) <= 1:
            return
        if inst.engine not in _ENGINE_OK:
            return
        waits = list(si.on_wait)
        del si.on_wait[1:]
        keep = si.on_wait[0]
        spill = [w for w in waits if w is not keep]
        for w in spill:
            nop = mybir.InstNoOp(
                name=f"I-{self.nc.next_id()}",
                engine=inst.engine,
                ins=[],
                outs=[],
                sync_info=mybir.SyncInfo(on_wait=[w], on_update=[]),
            )
            self._add_instruction(nop)

    def _commit_instruction(self, inst, lazy_reg_writes=True):
        if not (
            lazy_reg_writes
            and bass.is_reorderable_reg_write_inst(inst)
            and not (inst.sync_info and inst.sync_info.on_wait)
        ):
            self._spill_waits(inst)
        super()._commit_instruction(inst, lazy_reg_writes=lazy_reg_writes)

    def _drain_and_barrier(self, tick_clock, wait_clock):
        probe = self.nc.sync.nop()
        wait_clock.add_sem_waits(
            probe.ins, ScopedClock({None: tick_clock.global_clock})
        )
        si = probe.ins.sync_info
        waits = list(si.on_wait) if si is not None else []
        if si is not None and len(waits) > 1:
            del si.on_wait[1:]
            for w in waits[1:]:
                n = self.nc.sync.nop()
                nsi = n.ins.sync_info
                if nsi is None:
                    n.ins.sync_info = mybir.SyncInfo(on_wait=[w], on_update=[])
                else:
                    nsi.on_wait.append(w)
        self.nc.sync.drain()

        self.nc.all_engine_barrier()
        assert self.sems is not None
        popped = self.nc._tile_sem_poison_stack.pop()
        assert popped is self._sem_poison
        self.nc.clear_and_free_semaphores(list(self.sems.allocated().values()))
        self.nc.all_engine_barrier()


def _emit(tc, xt, wq_t, wk_t, wv_t, wp_t, maskb, onesb, out):
    nc = tc.nc
    with ExitStack() as ctx:
        consts = ctx.enter_context(tc.tile_pool(name="consts", bufs=1))
        persist = ctx.enter_context(tc.tile_pool(name="persist", bufs=1))

        mask_sb = consts.tile([128, 896], F32)
        nc.sync.dma_start(out=mask_sb[:], in_=maskb[:])
        ones_sb = consts.tile([1, 64], F32)
        nc.vector.memset(ones_sb[:], 1.0)

        # persistent activations (per-partition bytes: 16+16+16.6+16 KB)
        qT = persist.tile([128, 2, SEQ], F32R)     # [64q rows x 2 heads] x 2 grp
        kT = persist.tile([128, 2, SEQ], F32R)
        v1 = persist.tile([128, SEQ_T, HEADS_PER_CORE * 65], F32R)  # v|1 per head
        yT = persist.tile([128, 2, SEQ], F32R)

        # ---------------- phase 1: qkv projection (transposed) -------------
        with tc.tile_pool(name="p1sb", bufs=1) as p1sb, \
             tc.tile_pool(name="p1ps", bufs=2, space="PSUM") as p1ps, \
             tc.tile_pool(name="p1psv", bufs=2, space="PSUM") as p1psv:
            xt_sb = p1sb.tile([128, KT, SEQ], F32R)
            nc.sync.dma_start(
                out=xt_sb[:], in_=xt.rearrange("(k p) s -> p k s", p=128)
            )
            wq_sb = p1sb.tile([128, KT, CH], F32R)
            wk_sb = p1sb.tile([128, KT, CH], F32R)
            wv_sb = p1sb.tile([128, KT, CH], F32R)
            nc.sync.dma_start(
                out=wq_sb[:], in_=wq_t.rearrange("(k p) c -> p k c", p=128)
            )
            nc.sync.dma_start(
                out=wk_sb[:], in_=wk_t.rearrange("(k p) c -> p k c", p=128)
            )
            nc.sync.dma_start(
                out=wv_sb[:], in_=wv_t.rearrange("(k p) c -> p k c", p=128)
            )

            # q^T and k^T: out rows = 2 heads x 64, psum [128, 512]
            for which, wsb, dst in ((0, wq_sb, qT), (1, wk_sb, kT)):
                for g in range(2):          # head-pair group
                    for nch in range(SEQ // 512):
                        p = p1ps.tile([128, 512], F32, tag="qkps")
                        for k in range(KT):
                            nc.tensor.matmul(
                                p[:],
                                wsb[:, k, g * 128:(g + 1) * 128],
                                xt_sb[:, k, nch * 512:(nch + 1) * 512],
                                start=(k == 0),
                                stop=(k == KT - 1),
                            )
                        dslice = dst[:, g, nch * 512:(nch + 1) * 512]
                        if which == 0:
                            nc.vector.tensor_scalar_mul(
                                out=dslice, in0=p[:], scalar1=SCALE
                            )
                        else:
                            nc.vector.tensor_copy(dslice, p[:])

            # v in natural layout + ones column per head (stride 65)
            nc.sync.dma_start(
                out=v1[:].rearrange("p s (h c) -> p s h c", c=65)[:, :, :, 64:65],
                in_=onesb[:],
            )
            for st in range(SEQ_T):
                p = p1psv.tile([128, CH], F32, tag="vps")
                for k in range(KT):
                    nc.tensor.matmul(
                        p[:],
                        xt_sb[:, k, st * 128:(st + 1) * 128],
                        wv_sb[:, k, :],
                        start=(k == 0),
                        stop=(k == KT - 1),
                    )
                nc.vector.tensor_copy(
                    v1[:].rearrange("p s (h c) -> p s h c", c=65)[:, st, :, 0:64],
                    p[:].rearrange("p (h c) -> p h c", c=64),
                )

        # ---------------- phase 2: causal attention per head ---------------
        with tc.tile_pool(name="att", bufs=3) as attp, \
             tc.tile_pool(name="rec", bufs=2) as recp, \
             tc.tile_pool(name="p2s", bufs=3, space="PSUM") as p2s, \
             tc.tile_pool(name="p2u", bufs=2, space="PSUM") as p2u:
            for h in range(HEADS_PER_CORE):
                g, r0 = h // 2, (h % 2) * 64
                qTh = qT[r0:r0 + 64, g, :]
                kTh = kT[r0:r0 + 64, g, :]
                for qc in range(SEQ // QCH):
                    pu = p2u.tile([65, QCH], F32, tag="pu")
                    nkt = 4 * (qc + 1)
                    for kt in range(nkt):
                        psv = p2s.tile([128, QCH], F32, tag="ps")
                        nc.tensor.matmul(
                            psv[:],
                            kTh[:, kt * 128:(kt + 1) * 128],
                            qTh[:, qc * QCH:(qc + 1) * QCH],
                            start=True,
                            stop=True,
                        )
                        d = kt - 4 * qc
                        if d >= 0:  # diagonal tile: causal mask add
                            off = 384 - 128 * d
                            nc.vector.tensor_add(
                                psv[:], psv[:], mask_sb[:, off:off + QCH]
                            )
                        att = attp.tile([128, QCH], F32R, tag="att")
                        nc.scalar.activation(att[:], psv[:], EXP)
                        nc.tensor.matmul(
                            pu[:],
                            v1[:, kt, h * 65:h * 65 + 65],
                            att[:],
                            start=(kt == 0),
                            stop=(kt == nkt - 1),
                        )
                    rec = recp.tile([1, QCH], F32, tag="rec")
                    nc.vector.reciprocal(rec[:], pu[64:65, :])
                    # broadcast recip across 64 partitions via PE ones-matmul
                    pbc = p2s.tile([64, QCH], F32, tag="pbc")
                    nc.tensor.matmul(
                        pbc[:], ones_sb[:], rec[:], start=True, stop=True
                    )
                    u_sb = recp.tile([64, QCH], F32, tag="usb")
                    nc.vector.tensor_copy(u_sb[:], pu[0:64, :])
                    nc.vector.tensor_mul(
                        yT[r0:r0 + 64, g, qc * QCH:(qc + 1) * QCH],
                        u_sb[:],
                        pbc[:],
                    )

        # ------------- phase 3: AllGather y^T, column-sliced proj ----------
        with tc.tile_pool(name="p3sb", bufs=1) as p3sb, \
             tc.tile_pool(name="outs", bufs=3) as outsp, \
             tc.tile_pool(name="p3ps", bufs=2, space="PSUM") as p3ps, \
             tc.tile_pool(name="dram", bufs=1, space="DRAM") as dram:
            y_loc = dram.tile([CH, SEQ], F32R)
            y_all = dram.tile([GROUP * CH, SEQ], F32R)
            for g in range(2):
                nc.sync.dma_start(
                    out=y_loc[g * 128:(g + 1) * 128, :], in_=yT[:, g, :]
                )
            nc.gpsimd.collective_compute(
                "AllGather",
                mybir.AluOpType.bypass,
                replica_groups=[[0, 1, 2, 3], [4, 5, 6, 7]],
                ins=[y_loc.opt()],
                outs=[y_all.opt()],
            )
            yf_sb = p3sb.tile([128, KT, SEQ], F32R)
            nc.sync.dma_start(
                out=yf_sb[:], in_=y_all[:].rearrange("(k p) s -> p k s", p=128)
            )
            wp_sb = p3sb.tile([128, KT, CH], F32R)
            nc.sync.dma_start(
                out=wp_sb[:], in_=wp_t.rearrange("(k p) c -> p k c", p=128)
            )
            for st in range(SEQ_T):
                p = p3ps.tile([128, CH], F32, tag="ops")
                for k in range(KT):
                    nc.tensor.matmul(
                        p[:],
                        yf_sb[:, k, st * 128:(st + 1) * 128],
                        wp_sb[:, k, :],
                        start=(k == 0),
                        stop=(k == KT - 1),
                    )
                o = outsp.tile([128, CH], F32, tag="ot")
                nc.vector.tensor_copy(o[:], p[:])
                nc.sync.dma_start(
                    out=out[st * 128:(st + 1) * 128, :], in_=o[:]
                )


_CACHE = {}


def _build():
    if "nc" in _CACHE:
        return _CACHE["nc"]
    nc = bass.Bass("TRN2", target_bir_lowering=False, debug=False,
                   num_devices=N_CORES)
    xt = nc.dram_tensor("xt", [N_EMBD, SEQ], F32R, kind="ExternalInput").ap()
    wq_t = nc.dram_tensor("wq_t", [N_EMBD, CH], F32R, kind="ExternalInput").ap()
    wk_t = nc.dram_tensor("wk_t", [N_EMBD, CH], F32R, kind="ExternalInput").ap()
    wv_t = nc.dram_tensor("wv_t", [N_EMBD, CH], F32R, kind="ExternalInput").ap()
    wp_t = nc.dram_tensor("wp_t", [N_EMBD, CH], F32R, kind="ExternalInput").ap()
    maskb = nc.dram_tensor("maskb", [128, 896], F32, kind="ExternalInput").ap()
    onesb = nc.dram_tensor("onesb", [128, SEQ_T, HEADS_PER_CORE, 1], F32R,
                           kind="ExternalInput").ap()
    out = nc.dram_tensor("out", [SEQ, CH], F32, kind="ExternalOutput").ap()
    with SafeTileContext(nc) as tc:
        _emit(tc, xt, wq_t, wk_t, wv_t, wp_t, maskb, onesb, out)
    _CACHE["nc"] = nc
    return nc


def _get_executor():
    """Compile the SPMD program into a reusable jitted callable (no
    donation, so it can be invoked repeatedly for timing)."""
    if "exec" in _CACHE:
        return _CACHE["exec"]
    import jax
    from jax.sharding import Mesh, PartitionSpec
    from jax.experimental.shard_map import shard_map
    from concourse import bass2jax

    nc = _build()
    bass2jax.install_neuronx_cc_hook()
    pname = nc.partition_id_tensor.name if nc.partition_id_tensor else None
    in_names, out_names, out_avals, zero_outs = [], [], [], []
    for alloc in nc.m.functions[0].allocations:
        if not isinstance(alloc, mybir.MemoryLocationSet):
            continue
        name = alloc.memorylocations[0].name
        if alloc.kind == "ExternalInput":
            if name != pname:
                in_names.append(name)
        elif alloc.kind == "ExternalOutput":
            out_names.append(name)
            shape = tuple(alloc.tensor_shape)
            dtype = mybir.dt.np(alloc.dtype)
            out_avals.append(jax.core.ShapedArray(shape, dtype))
            zero_outs.append(np.zeros(shape, dtype))
    all_in = in_names + out_names + ([pname] if pname else [])

    def _body(*args):
        operands = list(args)
        if pname:
            operands.append(bass2jax.partition_id_tensor())
        outs = bass2jax._bass_exec_p.bind(
            *operands,
            out_avals=tuple(out_avals),
            in_names=tuple(all_in),
            out_names=tuple(out_names),
            lowering_input_output_aliases=(),
            sim_require_finite=True,
            sim_require_nnan=True,
            nc=nc,
        )
        return tuple(outs)

    devices = jax.devices()[:N_CORES]
    mesh = Mesh(np.asarray(devices), ("core",))
    nin = len(in_names) + len(out_names)
    f = jax.jit(
        shard_map(
            _body,
            mesh=mesh,
            in_specs=(PartitionSpec("core"),) * nin,
            out_specs=(PartitionSpec("core"),) * len(out_names),
            check_rep=False,
        ),
        keep_unused=True,
    )
    _CACHE["exec"] = (f, in_names, out_names, zero_outs)
    return _CACHE["exec"]


def _make_mask():
    i = np.arange(128, dtype=np.int64)[:, None]
    c = np.arange(896, dtype=np.int64)[None, :]
    return np.where(i <= c - 384, 0.0, NEG).astype(np.float32)


def _in_maps(x, w_qkv, w_proj):
    maskb = _make_mask()
    ones_col = np.ones((128, SEQ_T, HEADS_PER_CORE, 1), np.float32)
    maps = []
    for c in range(N_CORES):
        b, hb = c // GROUP, c % GROUP
        cs = slice(hb * CH, (hb + 1) * CH)
        maps.append({
            "xt": np.ascontiguousarray(x[b].T),
            "wq_t": np.ascontiguousarray(w_qkv[0 * N_EMBD:1 * N_EMBD][cs].T),
            "wk_t": np.ascontiguousarray(w_qkv[1 * N_EMBD:2 * N_EMBD][cs].T),
            "wv_t": np.ascontiguousarray(w_qkv[2 * N_EMBD:3 * N_EMBD][cs].T),
            "wp_t": np.ascontiguousarray(w_proj[cs, :].T),
            "maskb": maskb,
            "onesb": ones_col,
        })
    return maps


def _device_inputs(maps):
    import jax
    f, in_names, out_names, zero_outs = _get_executor()
    concat = [
        np.concatenate([maps[c][n] for c in range(N_CORES)], axis=0)
        for n in in_names
    ]
    concat += [
        np.concatenate([z] * N_CORES, axis=0) for z in zero_outs
    ]
    return [jax.device_put(a) for a in concat]


def _execute(dev_in):
    import jax
    f = _get_executor()[0]
    r = f(*dev_in)
    jax.block_until_ready(r)
    return r


def kernel(x, w_qkv, w_proj):
    x = np.asarray(x, np.float32)
    w_qkv = np.asarray(w_qkv, np.float32)
    w_proj = np.asarray(w_proj, np.float32)
    dev_in = _device_inputs(_in_maps(x, w_qkv, w_proj))
    _CACHE["dev_in"] = dev_in
    r = _execute(dev_in)
    res = np.asarray(r[0])          # [8*SEQ, CH]
    out = np.empty((BSZ, SEQ, N_EMBD), np.float32)
    for c in range(N_CORES):
        b, hb = c // GROUP, c % GROUP
        out[b, :, hb * CH:(hb + 1) * CH] = res[c * SEQ:(c + 1) * SEQ]
    return out


def bench(n=20):
    """Re-execute the last kernel() invocation n times; returns wall
    seconds per call (device inputs cached, jit warm)."""
    import time
    dev_in = _CACHE["dev_in"]
    _execute(dev_in)
    ts = []
    for _ in range(n):
        t0 = time.perf_counter()
        _execute(dev_in)
        ts.append(time.perf_counter() - t0)
    return np.array(ts)


# revision 24
# speedup vs baseline: 1.0004x; 1.0004x over previous
"""Causal self-attention (dense transformer block) on 8 Trainium2 NeuronCores.

Sharding: 2 batch groups x 4 cores. Within a group each core owns 4 heads
(tensor parallel) for qkv+attention, then an AllGather of y^T inside the
group lets each core compute a disjoint 256-column slice of the output
projection (column-parallel proj => no rank-dependent addressing needed).

x:      [2, 2048, 1024] f32
w_qkv:  [3072, 1024]    f32   (rows: q 0:1024, k 1024:2048, v 2048:3072)
w_proj: [1024, 1024]    f32
out:    [2, 2048, 1024] f32
"""

import sys

if "/opt/trn_rl_repo" not in sys.path:
    sys.path.insert(0, "/opt/trn_rl_repo")

from contextlib import ExitStack

import numpy as np

import concourse.bass as bass
import concourse.mybir as mybir
import concourse.tile as tile
from concourse.bass_utils import run_bass_kernel_spmd
from concourse.vector_clock import ScopedClock

F32 = mybir.dt.float32
F32R = mybir.dt.float32r
EXP = mybir.ActivationFunctionType.Exp

N_EMBD = 1024
SEQ = 2048
BSZ = 2
N_CORES = 8
GROUP = 4                 # cores per batch group
HEADS_PER_CORE = 4
HEAD_DIM = 64
CH = HEADS_PER_CORE * HEAD_DIM   # 256 channels per core
KT = N_EMBD // 128        # 8 contraction tiles over embd
SEQ_T = SEQ // 128        # 16 seq tiles
QCH = 512                 # q chunk (free dim of S^T matmuls)
NEG = -1.0e30
SCALE = 1.0 / 8.0         # 1/sqrt(64)


_ENGINE_OK = {
    mybir.EngineType.PE,
    mybir.EngineType.DVE,
    mybir.EngineType.Activation,
    mybir.EngineType.Pool,
    mybir.EngineType.SP,
}


class SafeTileContext(tile.TileContext):
    """This walrus build accepts only a single sync-wait per TPB engine
    instruction; Tile's add_semaphores attaches every required wait to the
    consuming instruction. Spill excess waits onto same-engine NOPs placed
    immediately before the instruction (engine program order preserves
    semantics). DMACopy is exempt (DGE-ring lowering handles multi-wait)."""

    def _spill_waits(self, inst):
        si = inst.sync_info
        if si is None or len(si.on_wait) <= 1:
            return
        if inst.engine not in _ENGINE_OK:
            return
        waits = list(si.on_wait)
        del si.on_wait[1:]
        keep = si.on_wait[0]
        spill = [w for w in waits if w is not keep]
        for w in spill:
            nop = mybir.InstNoOp(
                name=f"I-{self.nc.next_id()}",
                engine=inst.engine,
                ins=[],
                outs=[],
                sync_info=mybir.SyncInfo(on_wait=[w], on_update=[]),
            )
            self._add_instruction(nop)

    def _commit_instruction(self, inst, lazy_reg_writes=True):
        if not (
            lazy_reg_writes
            and bass.is_reorderable_reg_write_inst(inst)
            and not (inst.sync_info and inst.sync_info.on_wait)
        ):
            self._spill_waits(inst)
        super()._commit_instruction(inst, lazy_reg_writes=lazy_reg_writes)

    def _drain_and_barrier(self, tick_clock, wait_clock):
        probe = self.nc.sync.nop()
        wait_clock.add_sem_waits(
            probe.ins, ScopedClock({None: tick_clock.global_clock})
        )
        si = probe.ins.sync_info
        waits = list(si.on_wait) if si is not None else []
        if si is not None and len(waits) > 1:
            del si.on_wait[1:]
            for w in waits[1:]:
                n = self.nc.sync.nop()
                nsi = n.ins.sync_info
                if nsi is None:
                    n.ins.sync_info = mybir.SyncInfo(on_wait=[w], on_update=[])
                else:
                    nsi.on_wait.append(w)
        self.nc.sync.drain()

        self.nc.all_engine_barrier()
        assert self.sems is not None
        popped = self.nc._tile_sem_poison_stack.pop()
        assert popped is self._sem_poison
        self.nc.clear_and_free_semaphores(list(self.sems.allocated().values()))
        self.nc.all_engine_barrier()


def _emit(tc, xt, wq_t, wk_t, wv_t, wp_t, maskb, onesb, out):
    nc = tc.nc
    NQC = SEQ // QCH  # 4 q-chunks
    with ExitStack() as ctx:
        consts = ctx.enter_context(tc.tile_pool(name="consts", bufs=1))
        persist = ctx.enter_context(tc.tile_pool(name="persist", bufs=1))
        p1sb = ctx.enter_context(tc.tile_pool(name="p1sb", bufs=1))
        attp = ctx.enter_context(tc.tile_pool(name="att", bufs=5))
        recp = ctx.enter_context(tc.tile_pool(name="rec", bufs=2))
        yfp = ctx.enter_context(tc.tile_pool(name="yfp", bufs=2))
        outsp = ctx.enter_context(tc.tile_pool(name="outs", bufs=3))
        dram = ctx.enter_context(tc.tile_pool(name="dram", bufs=1, space="DRAM"))
        # single PSUM pool, 8 banks total:
        #   acc (qkv accum + proj out) x3, ps (scores + bcast) x3, pu x2
        psum = ctx.enter_context(tc.tile_pool(name="psum", bufs=1, space="PSUM"))

        mask_sb = consts.tile([128, 896], F32)
        nc.sync.dma_start(out=mask_sb[:], in_=maskb[:])
        ones1 = consts.tile([128, SEQ_T, HEADS_PER_CORE, 1], F32R)
        nc.sync.dma_start(out=ones1[:], in_=onesb[:])

        # persistent activations, split per chunk for fine-grained deps
        qTc = [persist.tile([128, 2, QCH], F32R, tag=f"qT{i}", name=f"qT{i}")
               for i in range(NQC)]
        kTc = [persist.tile([128, 2, QCH], F32R, tag=f"kT{i}", name=f"kT{i}")
               for i in range(NQC)]
        v1s = [persist.tile([128, HEADS_PER_CORE * 65], F32R, tag=f"v1{i}",
                            name=f"v1{i}") for i in range(SEQ_T)]
        yTc = [persist.tile([128, 2, QCH], F32R, tag=f"yT{i}", name=f"yT{i}")
               for i in range(NQC)]

        # inputs (xt streamed per q-chunk inside the main loop)
        wq_sb = p1sb.tile([128, KT, CH], F32R)
        wk_sb = p1sb.tile([128, KT, CH], F32R)
        wv_sb = p1sb.tile([128, KT, CH], F32R)
        wp_sb = p1sb.tile([128, KT, CH], F32R)
        nc.sync.dma_start(
            out=wq_sb[:], in_=wq_t.rearrange("(k p) c -> p k c", p=128)
        )
        xtc0 = []
        for k in range(KT):
            t = p1sb.tile([128, 512], F32R, tag=f"xt{k}", name=f"xt{k}", bufs=2)
            nc.sync.dma_start(out=t[:], in_=xt[k * 128:(k + 1) * 128, 0:512])
            xtc0.append(t)
        nc.sync.dma_start(
            out=wk_sb[:], in_=wk_t.rearrange("(k p) c -> p k c", p=128)
        )
        nc.sync.dma_start(
            out=wv_sb[:], in_=wv_t.rearrange("(k p) c -> p k c", p=128)
        )

        ones64 = ones1[0:1].rearrange("p s h o -> p (s h o)")  # [1, 64]

        for qc in range(NQC):
            # ---------------- qkv for this chunk -------------------------
            if qc == 0:
                xtc = xtc0
            else:
                xtc = []
                for k in range(KT):
                    t = p1sb.tile([128, 512], F32R, tag=f"xt{k}",
                                  name=f"xt{k}", bufs=2)
                    nc.sync.dma_start(
                        out=t[:],
                        in_=xt[k * 128:(k + 1) * 128, qc * 512:(qc + 1) * 512],
                    )
                    xtc.append(t)
            for which, wsb, dstc in ((0, wq_sb, qTc), (1, wk_sb, kTc)):
                for g in range(2):
                    p = psum.tile([128, 512], F32, tag="acc", name="acc", bufs=2)
                    for k in range(KT):
                        nc.tensor.matmul(
                            p[:],
                            wsb[:, k, g * 128:(g + 1) * 128],
                            xtc[k][:],
                            start=(k == 0),
                            stop=(k == KT - 1),
                        )
                    dslice = dstc[qc][:, g, :]
                    if which == 0:
                        nc.vector.tensor_scalar_mul(
                            out=dslice, in0=p[:], scalar1=SCALE
                        )
                    else:
                        nc.vector.tensor_copy(dslice, p[:])
            for sti in range(4):
                st = qc * 4 + sti
                p = psum.tile([128, CH], F32, tag="acc", name="acc", bufs=2)
                for k in range(KT):
                    nc.tensor.matmul(
                        p[:],
                        xtc[k][:, sti * 128:(sti + 1) * 128],
                        wv_sb[:, k, :],
                        start=(k == 0),
                        stop=(k == KT - 1),
                    )
                v1v = v1s[st][:].rearrange("p (h c) -> p h c", c=65)
                nc.vector.tensor_copy(
                    v1v[:, :, 0:64],
                    p[:].rearrange("p (h c) -> p h c", c=64),
                )
                nc.vector.tensor_copy(v1v[:, :, 64:65], ones1[:, st])

            # ---------------- attention for this chunk -------------------
            nkt = 4 * (qc + 1)
            for h in range(HEADS_PER_CORE):
                g, r0 = h // 2, (h % 2) * 64
                pu = psum.tile([65, QCH], F32, tag="pu", name="pu", bufs=2)
                for kp in range(nkt // 2):  # k-tile pairs share one psum+exp
                    psv = psum.tile([128, 2 * QCH], F32, tag="ps", name="ps",
                                    bufs=2)
                    att = attp.tile([128, 2 * QCH], F32R, tag="att")
                    c0s = []
                    for half in range(2):
                        kt = 2 * kp + half
                        d = kt - 4 * qc
                        c0 = 0 if d < 0 else min(128 * d, 256)
                        c0s.append(c0)
                        w = QCH - c0
                        kk = (kt % 4) * 128
                        base = half * QCH
                        nc.tensor.matmul(
                            psv[:, base + c0:base + QCH],
                            kTc[kt // 4][r0:r0 + 64, g, kk:kk + 128],
                            qTc[qc][r0:r0 + 64, g, c0:],
                            start=True,
                            stop=True,
                        )
                        if d >= 0:  # diagonal tile: causal mask add
                            off = 384 - 128 * d + c0
                            nc.vector.tensor_add(
                                psv[:, base + c0:base + QCH],
                                psv[:, base + c0:base + QCH],
                                mask_sb[:, off:off + w],
                            )
                    # one exp over the pair span; columns no matmul wrote are
                    # never read back (U slices skip them)
                    e0 = c0s[0]
                    nc.scalar.activation(att[:, e0:], psv[:, e0:], EXP)
                    for half in range(2):
                        kt = 2 * kp + half
                        c0 = c0s[half]
                        base = half * QCH
                        nc.tensor.matmul(
                            pu[:, c0:],
                            v1s[kt][:, h * 65:h * 65 + 65],
                            att[:, base + c0:base + QCH],
                            start=(kt == 0),
                            stop=(kt == nkt - 1),
                        )
                u_sb = recp.tile([65, QCH], F32, tag="usb")
                nc.vector.tensor_copy(u_sb[:], pu[:])  # frees pu for next head
                rec = recp.tile([1, QCH], F32R, tag="rec")
                with nc.allow_low_precision(reason="f32r normalization"):
                    nc.vector.reciprocal(rec[:], u_sb[64:65, :])
                bc_sb = recp.tile([64, QCH], F32R, tag="bc")
                nc.gpsimd.partition_broadcast(bc_sb[:], rec[:])
                nc.vector.tensor_mul(
                    yTc[qc][r0:r0 + 64, g, :],
                    u_sb[0:64, :],
                    bc_sb[:],
                )

            # -------- chunk complete: per-half AG (each launches once its
            # two heads finish) + proj ------------------------------------
            y_alls = []
            for g in range(2):
                y_loc = dram.tile([128, QCH], F32R, tag=f"yloc{qc}_{g}",
                                  name=f"yloc{qc}_{g}")
                y_all = dram.tile([GROUP * 128, QCH], F32R, tag=f"yall{qc}_{g}",
                                  name=f"yall{qc}_{g}")
                nc.sync.dma_start(out=y_loc[:], in_=yTc[qc][:, g, :])
                nc.gpsimd.collective_compute(
                    "AllGather",
                    mybir.AluOpType.bypass,
                    replica_groups=[[0, 1, 2, 3], [4, 5, 6, 7]],
                    ins=[y_loc.opt()],
                    outs=[y_all.opt()],
                )
                y_alls.append(y_all)
            if qc == 0:
                nc.sync.dma_start(
                    out=wp_sb[:], in_=wp_t.rearrange("(k p) c -> p k c", p=128)
                )
            yfs = []
            for k in range(KT):
                r, g = k // 2, k % 2  # global channel tile k = rank r, half g
                t = yfp.tile([128, QCH], F32R, tag=f"yf{k}", name=f"yf{k}")
                nc.sync.dma_start(
                    out=t[:], in_=y_alls[g][r * 128:(r + 1) * 128, :]
                )
                yfs.append(t)
            for sti in range(QCH // 128):
                st = qc * (QCH // 128) + sti
                p = psum.tile([128, CH], F32, tag="po", name="po", bufs=1)
                for k in range(KT):
                    nc.tensor.matmul(
                        p[:],
                        yfs[k][:, sti * 128:(sti + 1) * 128],
                        wp_sb[:, k, :],
                        start=(k == 0),
                        stop=(k == KT - 1),
                    )
                o = outsp.tile([128, CH], F32, tag="ot")
                nc.vector.tensor_copy(o[:], p[:])
                nc.sync.dma_start(
                    out=out[st * 128:(st + 1) * 128, :], in_=o[:]
                )


_CACHE = {}


def _build():
    if "nc" in _CACHE:
        return _CACHE["nc"]
    nc = bass.Bass("TRN2", target_bir_lowering=False, debug=False,
                   num_devices=N_CORES)
    xt = nc.dram_tensor("xt", [N_EMBD, SEQ], F32R, kind="ExternalInput").ap()
    wq_t = nc.dram_tensor("wq_t", [N_EMBD, CH], F32R, kind="ExternalInput").ap()
    wk_t = nc.dram_tensor("wk_t", [N_EMBD, CH], F32R, kind="ExternalInput").ap()
    wv_t = nc.dram_tensor("wv_t", [N_EMBD, CH], F32R, kind="ExternalInput").ap()
    wp_t = nc.dram_tensor("wp_t", [N_EMBD, CH], F32R, kind="ExternalInput").ap()
    maskb = nc.dram_tensor("maskb", [128, 896], F32, kind="ExternalInput").ap()
    onesb = nc.dram_tensor("onesb", [128, SEQ_T, HEADS_PER_CORE, 1], F32R,
                           kind="ExternalInput").ap()
    out = nc.dram_tensor("out", [SEQ, CH], F32, kind="ExternalOutput").ap()
    with SafeTileContext(nc) as tc:
        _emit(tc, xt, wq_t, wk_t, wv_t, wp_t, maskb, onesb, out)
    _CACHE["nc"] = nc
    return nc


def _get_executor():
    """Compile the SPMD program into a reusable jitted callable (no
    donation, so it can be invoked repeatedly for timing)."""
    if "exec" in _CACHE:
        return _CACHE["exec"]
    import jax
    from jax.sharding import Mesh, PartitionSpec
    from jax.experimental.shard_map import shard_map
    from concourse import bass2jax

    nc = _build()
    bass2jax.install_neuronx_cc_hook()
    pname = nc.partition_id_tensor.name if nc.partition_id_tensor else None
    in_names, out_names, out_avals, zero_outs = [], [], [], []
    for alloc in nc.m.functions[0].allocations:
        if not isinstance(alloc, mybir.MemoryLocationSet):
            continue
        name = alloc.memorylocations[0].name
        if alloc.kind == "ExternalInput":
            if name != pname:
                in_names.append(name)
        elif alloc.kind == "ExternalOutput":
            out_names.append(name)
            shape = tuple(alloc.tensor_shape)
            dtype = mybir.dt.np(alloc.dtype)
            out_avals.append(jax.core.ShapedArray(shape, dtype))
            zero_outs.append(np.zeros(shape, dtype))
    all_in = in_names + out_names + ([pname] if pname else [])

    def _body(*args):
        operands = list(args)
        if pname:
            operands.append(bass2jax.partition_id_tensor())
        outs = bass2jax._bass_exec_p.bind(
            *operands,
            out_avals=tuple(out_avals),
            in_names=tuple(all_in),
            out_names=tuple(out_names),
            lowering_input_output_aliases=(),
            sim_require_finite=True,
            sim_require_nnan=True,
            nc=nc,
        )
        return tuple(outs)

    devices = jax.devices()[:N_CORES]
    mesh = Mesh(np.asarray(devices), ("core",))
    nin = len(in_names) + len(out_names)
    f = jax.jit(
        shard_map(
            _body,
            mesh=mesh,
            in_specs=(PartitionSpec("core"),) * nin,
            out_specs=(PartitionSpec("core"),) * len(out_names),
            check_rep=False,
        ),
        keep_unused=True,
    )
    _CACHE["exec"] = (f, in_names, out_names, zero_outs)
    return _CACHE["exec"]


def _make_mask():
    i = np.arange(128, dtype=np.int64)[:, None]
    c = np.arange(896, dtype=np.int64)[None, :]
    return np.where(i <= c - 384, 0.0, NEG).astype(np.float32)


def _in_maps(x, w_qkv, w_proj):
    maskb = _make_mask()
    ones_col = np.ones((128, SEQ_T, HEADS_PER_CORE, 1), np.float32)
    maps = []
    for c in range(N_CORES):
        b, hb = c // GROUP, c % GROUP
        cs = slice(hb * CH, (hb + 1) * CH)
        maps.append({
            "xt": np.ascontiguousarray(x[b].T),
            "wq_t": np.ascontiguousarray(w_qkv[0 * N_EMBD:1 * N_EMBD][cs].T),
            "wk_t": np.ascontiguousarray(w_qkv[1 * N_EMBD:2 * N_EMBD][cs].T),
            "wv_t": np.ascontiguousarray(w_qkv[2 * N_EMBD:3 * N_EMBD][cs].T),
            "wp_t": np.ascontiguousarray(w_proj[cs, :].T),
            "maskb": maskb,
            "onesb": ones_col,
        })
    return maps


def _device_inputs(maps):
    import jax
    f, in_names, out_names, zero_outs = _get_executor()
    concat = [
        np.concatenate([maps[c][n] for c in range(N_CORES)], axis=0)
        for n in in_names
    ]
    concat += [
        np.concatenate([z] * N_CORES, axis=0) for z in zero_outs
    ]
    return [jax.device_put(a) for a in concat]


def _execute(dev_in):
    import jax
    f = _get_executor()[0]
    r = f(*dev_in)
    jax.block_until_ready(r)
    return r


def kernel(x, w_qkv, w_proj):
    x = np.asarray(x, np.float32)
    w_qkv = np.asarray(w_qkv, np.float32)
    w_proj = np.asarray(w_proj, np.float32)
    dev_in = _device_inputs(_in_maps(x, w_qkv, w_proj))
    _CACHE["dev_in"] = dev_in
    r = _execute(dev_in)
    res = np.asarray(r[0])          # [8*SEQ, CH]
    out = np.empty((BSZ, SEQ, N_EMBD), np.float32)
    for c in range(N_CORES):
        b, hb = c // GROUP, c % GROUP
        out[b, :, hb * CH:(hb + 1) * CH] = res[c * SEQ:(c + 1) * SEQ]
    return out


def bench(n=20):
    """Re-execute the last kernel() invocation n times; returns wall
    seconds per call (device inputs cached, jit warm)."""
    import time
    dev_in = _CACHE["dev_in"]
    _execute(dev_in)
    ts = []
    for _ in range(n):
        t0 = time.perf_counter()
        _execute(dev_in)
        ts.append(time.perf_counter() - t0)
    return np.array(ts)
